# revision 5
# baseline (speedup 1.0000x reference)
"""Trainium2 Bass kernel for nn_MoE_77644418777543.

MoE: B=4096 tokens, D=512 in, H=1024 hidden, E=16 experts (dense compute,
top-4 weighted combine), gate = Linear+LN+GELU+Linear+softmax, final head
Linear+BN+GELU+Linear.

Strategy: data-parallel over batch across 8 NeuronCores (512 tokens/core),
expert/gate/final weights replicated. No collectives. Per core:
  - gate computed token-major in fp32 (routing selection must be exact);
  - expert MLPs computed feature-major with fp32r matmuls (full PE rate,
    ~1.5e-4 precision) with BN+bias+GELU folded into ScalarE activation;
  - dense top-4 combine via per-token weight vector (zeros outside top-4)
    accumulated on VectorE;
  - final head feature-major fp32r.

Host-side prep: concat+transpose of inputs, BN folding (float64), layout
rearrangement. Output is [20, 512] per core, transposed+stacked on host.
"""

import numpy as np

import concourse.bacc as bacc
import concourse.bass as bass
import concourse.mybir as mybir
import concourse.tile as tile
from concourse.bass_utils import run_bass_kernel_spmd
from concourse.masks import make_identity

B, D, H, E, C = 4096, 512, 1024, 16, 20
NCORES = 8
BS = B // NCORES  # 512 tokens per core
EPS = 1e-5

f32 = mybir.dt.float32
f32r = mybir.dt.float32r
AF = mybir.ActivationFunctionType
OP = mybir.AluOpType

DC = D // 128  # 4 contraction tiles for D
HC = H // 128  # 8 contraction tiles for H
BT = BS // 128  # 4 token tiles per core


def build_nc():
    nc = bacc.Bacc("TRN2", target_bir_lowering=False)

    # ---- DRAM I/O ----
    xT_d = nc.dram_tensor("xT", [D, BS], f32r, kind="ExternalInput")
    gw1_d = nc.dram_tensor("gw1", [D, H], f32, kind="ExternalInput")
    gb1_d = nc.dram_tensor("gb1", [H], f32, kind="ExternalInput")
    lng_d = nc.dram_tensor("lng", [H], f32, kind="ExternalInput")
    lnb_d = nc.dram_tensor("lnb", [H], f32, kind="ExternalInput")
    gw2_d = nc.dram_tensor("gw2", [H, E], f32, kind="ExternalInput")
    gb2_d = nc.dram_tensor("gb2", [E], f32, kind="ExternalInput")
    ew1_d = nc.dram_tensor("ew1", [E, D, H], f32r, kind="ExternalInput")
    ew2_d = nc.dram_tensor("ew2", [E, H, H], f32r, kind="ExternalInput")
    es1_d = nc.dram_tensor("es1", [E, 128, HC], f32, kind="ExternalInput")
    et1_d = nc.dram_tensor("et1", [E, 128, HC], f32, kind="ExternalInput")
    es2_d = nc.dram_tensor("es2", [E, 128, HC], f32, kind="ExternalInput")
    et2_d = nc.dram_tensor("et2", [E, 128, HC], f32, kind="ExternalInput")
    fw1_d = nc.dram_tensor("fw1", [H, 512], f32r, kind="ExternalInput")
    fs_d = nc.dram_tensor("fs", [128, 4], f32, kind="ExternalInput")
    ft_d = nc.dram_tensor("ft", [128, 4], f32, kind="ExternalInput")
    fw2_d = nc.dram_tensor("fw2", [512, C], f32r, kind="ExternalInput")
    fb2_d = nc.dram_tensor("fb2", [C, 1], f32, kind="ExternalInput")
    out_d = nc.dram_tensor("outT", [C, BS], f32, kind="ExternalOutput")

    def bcast_ap(handle):
        """AP reading a 1-D dram tensor broadcast across 128 partitions."""
        ap = handle[:]
        return bass.AP(tensor=ap.tensor, offset=ap.offset, ap=[[0, 128]] + ap.ap)

    with tile.TileContext(nc) as tc:
        with (
            # big streaming pool for expert weights (created first)
            tc.tile_pool(name="pw", bufs=2) as pw,
            # persistent singles
            tc.tile_pool(name="px", bufs=1) as px,
            # expert hidden (double-buffered across experts)
            tc.tile_pool(name="ph", bufs=2) as ph,
            # gate big working tiles
            tc.tile_pool(name="pgb", bufs=1) as pgb,
            # gate small working tiles
            tc.tile_pool(name="pgs", bufs=2) as pgs,
            # small transient tiles
            tc.tile_pool(name="pt", bufs=3) as pt,
            # PSUM pools
            tc.tile_pool(name="pmm", bufs=4, space="PSUM") as pmm,
            tc.tile_pool(name="pwb", bufs=2, space="PSUM") as pwb,
        ):
            # ---- persistent SBUF loads ----
            xT = px.tile([128, DC, BS], f32r)
            nc.sync.dma_start(xT, xT_d[:, :].rearrange("(dc p) b -> p dc b", p=128))
            xT32 = xT.bitcast(f32)

            gw1 = px.tile([128, DC, H], f32)
            nc.sync.dma_start(gw1, gw1_d[:, :].rearrange("(dc p) h -> p dc h", p=128))
            gw2 = px.tile([128, HC, E], f32)
            nc.sync.dma_start(gw2, gw2_d[:, :].rearrange("(kc p) e -> p kc e", p=128))

            gb1b = px.tile([128, H], f32)
            nc.gpsimd.dma_start(gb1b, bcast_ap(gb1_d))
            lngb = px.tile([128, H], f32)
            nc.gpsimd.dma_start(lngb, bcast_ap(lng_d))
            lnbb = px.tile([128, H], f32)
            nc.gpsimd.dma_start(lnbb, bcast_ap(lnb_d))
            gb2b = px.tile([128, E], f32)
            nc.gpsimd.dma_start(gb2b, bcast_ap(gb2_d))

            es1 = px.tile([128, E, HC], f32)
            nc.sync.dma_start(es1, es1_d[:, :, :].rearrange("e p m -> p e m"))
            et1 = px.tile([128, E, HC], f32)
            nc.sync.dma_start(et1, et1_d[:, :, :].rearrange("e p m -> p e m"))
            es2 = px.tile([128, E, HC], f32)
            nc.sync.dma_start(es2, es2_d[:, :, :].rearrange("e p m -> p e m"))
            et2 = px.tile([128, E, HC], f32)
            nc.sync.dma_start(et2, et2_d[:, :, :].rearrange("e p m -> p e m"))

            fw1 = px.tile([128, HC, 512], f32r)
            nc.sync.dma_start(fw1, fw1_d[:, :].rearrange("(kc p) m -> p kc m", p=128))
            fsb = px.tile([128, 4], f32)
            nc.sync.dma_start(fsb, fs_d[:, :])
            ftb = px.tile([128, 4], f32)
            nc.sync.dma_start(ftb, ft_d[:, :])
            fw2 = px.tile([128, 4, C], f32r)
            nc.sync.dma_start(fw2, fw2_d[:, :].rearrange("(kc p) c -> p kc c", p=128))
            fb2 = px.tile([C, 1], f32)
            nc.sync.dma_start(fb2, fb2_d[:, :])

            ident = px.tile([128, 128], f32)
            make_identity(nc, ident)
            ones_row = px.tile([1, 128], f32)
            nc.vector.memset(ones_row, 1.0)
            eps_t = px.tile([128, 1], f32)
            nc.vector.memset(eps_t, EPS)

            wT = px.tile([E, BS], f32)  # routing weights, expert-major
            fused = px.tile([128, HC, BS], f32)  # combined expert output

            # ================= GATE (token-major, fp32) =================
            with tc.tile_pool(name="pgp", bufs=2, space="PSUM") as pgp:
                for bt in range(BT):
                    bs = bass.ts(bt, 128)
                    # g1 = xT.T @ gw1 + gb1  -> [128 tok, 1024 h]
                    g1 = pgb.tile([128, H], f32, tag="g1")
                    for hh in range(2):
                        ps = pmm.tile([128, 512], f32, tag="mm")
                        for dc in range(DC):
                            nc.tensor.matmul(
                                ps,
                                xT32[:, dc, bs],
                                gw1[:, dc, bass.ts(hh, 512)],
                                start=(dc == 0),
                                stop=(dc == DC - 1),
                            )
                        nc.vector.tensor_add(
                            g1[:, bass.ts(hh, 512)], ps, gb1b[:, bass.ts(hh, 512)]
                        )
                    # LayerNorm over h (free dim), in place on g1
                    stats = pgs.tile([128, 2, 6], f32, tag="stats")
                    nc.vector.bn_stats(stats[:, 0, :], g1[:, 0:512])
                    nc.vector.bn_stats(stats[:, 1, :], g1[:, 512:1024])
                    mv = pgs.tile([128, 2], f32, tag="mv")
                    nc.vector.bn_aggr(mv, stats)
                    sd = pgs.tile([128, 1], f32, tag="sd")
                    nc.scalar.activation(sd, mv[:, 1:2], AF.Sqrt, bias=eps_t)
                    rstd = pgs.tile([128, 1], f32, tag="rstd")
                    nc.vector.reciprocal(rstd, sd)
                    nc.vector.tensor_scalar(
                        g1, g1, mv[:, 0:1], rstd, op0=OP.subtract, op1=OP.mult
                    )
                    # affine + gelu
                    nc.vector.tensor_mul(g1, g1, lngb)
                    nc.vector.tensor_add(g1, g1, lnbb)
                    g1n = pgb.tile([128, H], f32, tag="g1n")
                    nc.scalar.activation(g1n, g1, AF.Gelu)
                    # transpose g1n -> feature-major g1nT (this token tile only)
                    g1nT = pgb.tile([128, HC, 128], f32, tag="g1nT")
                    for hc in range(HC):
                        tp = pgp.tile([128, 128], f32, tag="gps")
                        nc.tensor.transpose(tp, g1n[:, bass.ts(hc, 128)], ident)
                        nc.scalar.copy(g1nT[:, hc, :], tp)
                    # g2 logits = g1n @ gw2 + gb2 -> [128 tok, 16]
                    ps2 = pgp.tile([128, E], f32, tag="gps")
                    for hc in range(HC):
                        nc.tensor.matmul(
                            ps2,
                            g1nT[:, hc, :],
                            gw2[:, hc, :],
                            start=(hc == 0),
                            stop=(hc == HC - 1),
                        )
                    logits = pgs.tile([128, E], f32, tag="logits")
                    nc.vector.tensor_add(logits, ps2, gb2b)
                    # softmax over 16 experts
                    negmax = pgs.tile([128, 1], f32, tag="negmax")
                    nc.vector.tensor_reduce(
                        negmax, logits, axis=mybir.AxisListType.X, op=OP.max, negate=True
                    )
                    pexp = pgs.tile([128, E], f32, tag="pexp")
                    nc.scalar.activation(pexp, logits, AF.Exp, bias=negmax)
                    rsum = pgs.tile([128, 1], f32, tag="rsum")
                    nc.vector.reduce_sum(rsum, pexp, axis=mybir.AxisListType.X)
                    rinv = pgs.tile([128, 1], f32, tag="rinv")
                    nc.vector.reciprocal(rinv, rsum)
                    probs = pgs.tile([128, E], f32, tag="probs")
                    nc.vector.tensor_scalar_mul(probs, pexp, rinv)
                    # top-4 mask + renormalized weights
                    top8 = pgs.tile([128, 8], f32, tag="top8")
                    nc.vector.max(top8, probs)
                    mask = pgs.tile([128, E], f32, tag="mask")
                    nc.vector.tensor_scalar(
                        mask, probs, top8[:, 3:4], None, op0=OP.is_ge
                    )
                    eprob = pgs.tile([128, E], f32, tag="eprob")
                    nc.scalar.activation(eprob, probs, AF.Exp)
                    emask = pgs.tile([128, E], f32, tag="emask")
                    nc.vector.tensor_mul(emask, eprob, mask)
                    den = pgs.tile([128, 1], f32, tag="den")
                    nc.vector.reduce_sum(den, emask, axis=mybir.AxisListType.X)
                    dinv = pgs.tile([128, 1], f32, tag="dinv")
                    nc.vector.reciprocal(dinv, den)
                    w = pgs.tile([128, E], f32, tag="w")
                    nc.vector.tensor_scalar_mul(w, emask, dinv)
                    # transpose w -> wT[:, bt]
                    tpw = pgp.tile([E, 128], f32, tag="gps")
                    nc.tensor.transpose(tpw, w, ident)
                    nc.scalar.copy(wT[:, bs], tpw)

            # ================= EXPERTS (feature-major, fp32r) =================
            for e in range(E):
                h1T = ph.tile([128, HC, BS], f32r, tag="h1T")
                # layer 1: h1 = gelu(s1*(xT.T@w1) + t1), [1024 h, 512 tok]
                for q in range(4):
                    w1q = pw.tile([128, DC, 256], f32r, tag="ew1q")
                    nc.sync.dma_start(
                        w1q,
                        ew1_d[e, :, bass.ts(q, 256)].rearrange(
                            "(dc p) m -> p dc m", p=128
                        ),
                    )
                    for mi in range(2):
                        hidx = q * 2 + mi
                        ps = pmm.tile([128, BS], f32, tag="mm")
                        for dc in range(DC):
                            nc.tensor.matmul(
                                ps,
                                w1q[:, dc, bass.ts(mi, 128)],
                                xT[:, dc, :],
                                start=(dc == 0),
                                stop=(dc == DC - 1),
                            )
                        nc.scalar.activation(
                            h1T[:, hidx, :],
                            ps,
                            AF.Gelu,
                            bias=et1[:, e, hidx : hidx + 1],
                            scale=es1[:, e, hidx : hidx + 1],
                        )
                # routing weight broadcast: wb[p, b] = wT[e, b]
                wrow = pt.tile([1, BS], f32, tag="wrow")
                nc.sync.dma_start(wrow, wT[e : e + 1, :])
                wb = pwb.tile([128, BS], f32, tag="wb")
                nc.tensor.matmul(wb, ones_row[0:1, :], wrow[0:1, :], start=True, stop=True)
                # layer 2 + combine
                for q in range(4):
                    w2q = pw.tile([128, HC, 256], f32r, tag="ew2q")
                    nc.sync.dma_start(
                        w2q,
                        ew2_d[e, :, bass.ts(q, 256)].rearrange(
                            "(kc p) m -> p kc m", p=128
                        ),
                    )
                    for mi in range(2):
                        m2 = q * 2 + mi
                        ps = pmm.tile([128, BS], f32, tag="mm")
                        for kc in range(HC):
                            nc.tensor.matmul(
                                ps,
                                w2q[:, kc, bass.ts(mi, 128)],
                                h1T[:, kc, :],
                                start=(kc == 0),
                                stop=(kc == HC - 1),
                            )
                        h2 = pt.tile([128, BS], f32, tag="h2")
                        nc.scalar.activation(
                            h2,
                            ps,
                            AF.Gelu,
                            bias=et2[:, e, m2 : m2 + 1],
                            scale=es2[:, e, m2 : m2 + 1],
                        )
                        if e == 0:
                            nc.vector.tensor_mul(fused[:, m2, :], h2, wb)
                        else:
                            tmp = pt.tile([128, BS], f32, tag="cmb")
                            nc.vector.tensor_mul(tmp, h2, wb)
                            nc.vector.tensor_add(
                                fused[:, m2, :], fused[:, m2, :], tmp
                            )

            # ================= FINAL HEAD (feature-major, fp32r) =================
            fusedr = ph.tile([128, HC, BS], f32r, tag="h1T")
            nc.vector.tensor_copy(fusedr, fused)
            ffT = ph.tile([128, 4, BS], f32r, tag="h1T")
            for m in range(4):
                ps = pmm.tile([128, BS], f32, tag="mm")
                for kc in range(HC):
                    nc.tensor.matmul(
                        ps,
                        fw1[:, kc, bass.ts(m, 128)],
                        fusedr[:, kc, :],
                        start=(kc == 0),
                        stop=(kc == HC - 1),
                    )
                nc.scalar.activation(
                    ffT[:, m, :],
                    ps,
                    AF.Gelu,
                    bias=ftb[:, m : m + 1],
                    scale=fsb[:, m : m + 1],
                )
            with tc.tile_pool(name="pfp", bufs=2, space="PSUM") as pfp:
                ps = pfp.tile([C, BS], f32, tag="fo")
                for kc in range(4):
                    nc.tensor.matmul(
                        ps,
                        fw2[:, kc, :],
                        ffT[:, kc, :],
                        start=(kc == 0),
                        stop=(kc == 3),
                    )
                outT = pt.tile([C, BS], f32, tag="outT")
                nc.scalar.activation(outT, ps, AF.Identity, bias=fb2[:, 0:1])
                nc.sync.dma_start(out_d[:, :], outT)

    nc.compile()
    return nc


def prep_inputs(inputs):
    """Host-side prep: returns list of per-core input maps."""
    g = {k: np.asarray(v, dtype=np.float32) for k, v in inputs.items()}

    combined = np.concatenate([g["wifi_feat"], g["rfid_feat"]], axis=1)  # [B, D]
    xT = np.ascontiguousarray(combined.T)  # [D, B]

    def fold(b_lin, bn_g, bn_b, bn_m, bn_v):
        s = bn_g.astype(np.float64) / np.sqrt(bn_v.astype(np.float64) + EPS)
        t = (b_lin.astype(np.float64) - bn_m.astype(np.float64)) * s + bn_b.astype(
            np.float64
        )
        return s.astype(np.float32), t.astype(np.float32)

    s1, t1 = fold(g["exp_b1"], g["exp_bn1_g"], g["exp_bn1_b"], g["exp_bn1_m"], g["exp_bn1_v"])
    s2, t2 = fold(g["exp_b2"], g["exp_bn2_g"], g["exp_bn2_b"], g["exp_bn2_m"], g["exp_bn2_v"])
    fs, ft = fold(g["fin_b1"], g["fin_bn_g"], g["fin_bn_b"], g["fin_bn_m"], g["fin_bn_v"])

    def pmaj(x):  # [..., M*128] -> [..., 128, M] partition-major
        return np.ascontiguousarray(
            x.reshape(*x.shape[:-1], x.shape[-1] // 128, 128).swapaxes(-1, -2)
        )

    shared = {
        "gw1": np.ascontiguousarray(g["gate_w1"]),
        "gb1": g["gate_b1"],
        "lng": g["gate_ln_g"],
        "lnb": g["gate_ln_b"],
        "gw2": np.ascontiguousarray(g["gate_w2"]),
        "gb2": g["gate_b2"],
        "ew1": np.ascontiguousarray(g["exp_w1"]),
        "ew2": np.ascontiguousarray(g["exp_w2"]),
        "es1": pmaj(s1),
        "et1": pmaj(t1),
        "es2": pmaj(s2),
        "et2": pmaj(t2),
        "fw1": np.ascontiguousarray(g["fin_w1"]),
        "fs": pmaj(fs),
        "ft": pmaj(ft),
        "fw2": np.ascontiguousarray(g["fin_w2"]),
        "fb2": np.ascontiguousarray(g["fin_b2"].reshape(C, 1)),
    }
    per_core = []
    for c in range(NCORES):
        m = dict(shared)
        m["xT"] = np.ascontiguousarray(xT[:, c * BS : (c + 1) * BS])
        per_core.append(m)
    return per_core


_NC_CACHE = None


def kernel(**inputs) -> np.ndarray:
    global _NC_CACHE
    if _NC_CACHE is None:
        _NC_CACHE = build_nc()
    nc = _NC_CACHE
    in_maps = prep_inputs(inputs)
    res = run_bass_kernel_spmd(nc, in_maps, core_ids=list(range(NCORES)))
    out = np.concatenate(
        [np.asarray(r["outT"]).T for r in res.results], axis=0
    )  # [B, C]
    return np.ascontiguousarray(out)


# revision 6
# speedup vs baseline: 1.0795x; 1.0795x over previous
"""Trainium2 Bass kernel for nn_MoE_77644418777543.

MoE: B=4096 tokens, D=512 in, H=1024 hidden, E=16 experts (dense compute,
top-4 weighted combine), gate = Linear+LN+GELU+Linear+softmax, final head
Linear+BN+GELU+Linear.

Strategy: data-parallel over batch across 8 NeuronCores (512 tokens/core),
expert/gate/final weights replicated. No collectives. Per core:
  - gate computed token-major in fp32 (routing selection must be exact);
  - expert MLPs computed feature-major with fp32r matmuls (full PE rate,
    ~1.5e-4 precision) with BN+bias+GELU folded into ScalarE activation;
  - dense top-4 combine via per-token weight vector (zeros outside top-4)
    accumulated on VectorE;
  - final head feature-major fp32r.

Host-side prep: concat+transpose of inputs, BN folding (float64), layout
rearrangement. Output is [20, 512] per core, transposed+stacked on host.
"""

import numpy as np

import concourse.bacc as bacc
import concourse.bass as bass
import concourse.mybir as mybir
import concourse.tile as tile
from concourse.bass_utils import run_bass_kernel_spmd
from concourse.masks import make_identity

B, D, H, E, C = 4096, 512, 1024, 16, 20
NCORES = 8
BS = B // NCORES  # 512 tokens per core
EPS = 1e-5

f32 = mybir.dt.float32
f32r = mybir.dt.float32r
AF = mybir.ActivationFunctionType
OP = mybir.AluOpType

DC = D // 128  # 4 contraction tiles for D
HC = H // 128  # 8 contraction tiles for H
BT = BS // 128  # 4 token tiles per core


def build_nc():
    nc = bacc.Bacc("TRN2", target_bir_lowering=False)

    # ---- DRAM I/O ----
    xT_d = nc.dram_tensor("xT", [D, BS], f32r, kind="ExternalInput")
    gw1_d = nc.dram_tensor("gw1", [D, H], f32, kind="ExternalInput")
    gb1_d = nc.dram_tensor("gb1", [H], f32, kind="ExternalInput")
    lng_d = nc.dram_tensor("lng", [H], f32, kind="ExternalInput")
    lnb_d = nc.dram_tensor("lnb", [H], f32, kind="ExternalInput")
    gw2_d = nc.dram_tensor("gw2", [H, E], f32, kind="ExternalInput")
    gb2_d = nc.dram_tensor("gb2", [E], f32, kind="ExternalInput")
    ew1_d = nc.dram_tensor("ew1", [E, D, H], f32r, kind="ExternalInput")
    ew2_d = nc.dram_tensor("ew2", [E, H, H], f32r, kind="ExternalInput")
    es1_d = nc.dram_tensor("es1", [E, 128, HC], f32, kind="ExternalInput")
    et1_d = nc.dram_tensor("et1", [E, 128, HC], f32, kind="ExternalInput")
    es2_d = nc.dram_tensor("es2", [E, 128, HC], f32, kind="ExternalInput")
    et2_d = nc.dram_tensor("et2", [E, 128, HC], f32, kind="ExternalInput")
    fw1_d = nc.dram_tensor("fw1", [H, 512], f32r, kind="ExternalInput")
    fs_d = nc.dram_tensor("fs", [128, 4], f32, kind="ExternalInput")
    ft_d = nc.dram_tensor("ft", [128, 4], f32, kind="ExternalInput")
    fw2_d = nc.dram_tensor("fw2", [512, C], f32r, kind="ExternalInput")
    fb2_d = nc.dram_tensor("fb2", [C, 1], f32, kind="ExternalInput")
    out_d = nc.dram_tensor("outT", [C, BS], f32, kind="ExternalOutput")

    def bcast_ap(handle):
        """AP reading a 1-D dram tensor broadcast across 128 partitions."""
        ap = handle[:]
        return bass.AP(tensor=ap.tensor, offset=ap.offset, ap=[[0, 128]] + ap.ap)

    with tile.TileContext(nc) as tc:
        with (
            # big streaming pool for expert weights (created first)
            tc.tile_pool(name="pw", bufs=2) as pw,
            # persistent singles
            tc.tile_pool(name="px", bufs=1) as px,
            # expert hidden (double-buffered across experts)
            tc.tile_pool(name="ph", bufs=2) as ph,
            # gate big working tiles
            tc.tile_pool(name="pgb", bufs=1) as pgb,
            # gate small working tiles
            tc.tile_pool(name="pgs", bufs=2) as pgs,
            # small transient tiles
            tc.tile_pool(name="pt", bufs=3) as pt,
            # PSUM pools
            tc.tile_pool(name="pmm", bufs=4, space="PSUM") as pmm,
            tc.tile_pool(name="pwb", bufs=2, space="PSUM") as pwb,
        ):
            # ---- persistent SBUF loads ----
            xT = px.tile([128, DC, BS], f32r)
            nc.sync.dma_start(xT, xT_d[:, :].rearrange("(dc p) b -> p dc b", p=128))
            xT32 = xT.bitcast(f32)

            gw1 = px.tile([128, DC, H], f32)
            nc.sync.dma_start(gw1, gw1_d[:, :].rearrange("(dc p) h -> p dc h", p=128))
            gw2 = px.tile([128, HC, E], f32)
            nc.sync.dma_start(gw2, gw2_d[:, :].rearrange("(kc p) e -> p kc e", p=128))

            gb1b = px.tile([128, H], f32)
            nc.gpsimd.dma_start(gb1b, bcast_ap(gb1_d))
            lngb = px.tile([128, H], f32)
            nc.gpsimd.dma_start(lngb, bcast_ap(lng_d))
            lnbb = px.tile([128, H], f32)
            nc.gpsimd.dma_start(lnbb, bcast_ap(lnb_d))
            gb2b = px.tile([128, E], f32)
            nc.gpsimd.dma_start(gb2b, bcast_ap(gb2_d))

            es1 = px.tile([128, E, HC], f32)
            nc.sync.dma_start(es1, es1_d[:, :, :].rearrange("e p m -> p e m"))
            et1 = px.tile([128, E, HC], f32)
            nc.sync.dma_start(et1, et1_d[:, :, :].rearrange("e p m -> p e m"))
            es2 = px.tile([128, E, HC], f32)
            nc.sync.dma_start(es2, es2_d[:, :, :].rearrange("e p m -> p e m"))
            et2 = px.tile([128, E, HC], f32)
            nc.sync.dma_start(et2, et2_d[:, :, :].rearrange("e p m -> p e m"))

            fw1 = px.tile([128, HC, 512], f32r)
            nc.sync.dma_start(fw1, fw1_d[:, :].rearrange("(kc p) m -> p kc m", p=128))
            fsb = px.tile([128, 4], f32)
            nc.sync.dma_start(fsb, fs_d[:, :])
            ftb = px.tile([128, 4], f32)
            nc.sync.dma_start(ftb, ft_d[:, :])
            fw2 = px.tile([128, 4, C], f32r)
            nc.sync.dma_start(fw2, fw2_d[:, :].rearrange("(kc p) c -> p kc c", p=128))
            fb2 = px.tile([C, 1], f32)
            nc.sync.dma_start(fb2, fb2_d[:, :])

            ident = px.tile([128, 128], f32)
            make_identity(nc, ident)
            ones_row = px.tile([1, 128], f32)
            nc.vector.memset(ones_row, 1.0)
            eps_t = px.tile([128, 1], f32)
            nc.vector.memset(eps_t, EPS)

            wT = px.tile([E, BS], f32)  # routing weights, expert-major
            fused = px.tile([128, HC, BS], f32)  # combined expert output

            # ================= GATE (token-major, fp32) =================
            with tc.tile_pool(name="pgp", bufs=2, space="PSUM") as pgp:
                for bt in range(BT):
                    bs = bass.ts(bt, 128)
                    # g1 = xT.T @ gw1 + gb1  -> [128 tok, 1024 h]
                    g1 = pgb.tile([128, H], f32, tag="g1")
                    for hh in range(2):
                        ps = pmm.tile([128, 512], f32, tag="mm")
                        for dc in range(DC):
                            nc.tensor.matmul(
                                ps,
                                xT32[:, dc, bs],
                                gw1[:, dc, bass.ts(hh, 512)],
                                start=(dc == 0),
                                stop=(dc == DC - 1),
                            )
                        nc.vector.tensor_add(
                            g1[:, bass.ts(hh, 512)], ps, gb1b[:, bass.ts(hh, 512)]
                        )
                    # LayerNorm over h (free dim), in place on g1
                    stats = pgs.tile([128, 2, 6], f32, tag="stats")
                    nc.vector.bn_stats(stats[:, 0, :], g1[:, 0:512])
                    nc.vector.bn_stats(stats[:, 1, :], g1[:, 512:1024])
                    mv = pgs.tile([128, 2], f32, tag="mv")
                    nc.vector.bn_aggr(mv, stats)
                    sd = pgs.tile([128, 1], f32, tag="sd")
                    nc.scalar.activation(sd, mv[:, 1:2], AF.Sqrt, bias=eps_t)
                    rstd = pgs.tile([128, 1], f32, tag="rstd")
                    nc.vector.reciprocal(rstd, sd)
                    nc.vector.tensor_scalar(
                        g1, g1, mv[:, 0:1], rstd, op0=OP.subtract, op1=OP.mult
                    )
                    # affine + gelu
                    nc.vector.tensor_mul(g1, g1, lngb)
                    nc.vector.tensor_add(g1, g1, lnbb)
                    g1n = pgb.tile([128, H], f32, tag="g1n")
                    nc.scalar.activation(g1n, g1, AF.Gelu)
                    # transpose g1n -> feature-major g1nT (this token tile only)
                    g1nT = pgb.tile([128, HC, 128], f32, tag="g1nT")
                    for hc in range(HC):
                        tp = pgp.tile([128, 128], f32, tag="gps")
                        nc.tensor.transpose(tp, g1n[:, bass.ts(hc, 128)], ident)
                        nc.scalar.copy(g1nT[:, hc, :], tp)
                    # g2 logits = g1n @ gw2 + gb2 -> [128 tok, 16]
                    ps2 = pgp.tile([128, E], f32, tag="gps")
                    for hc in range(HC):
                        nc.tensor.matmul(
                            ps2,
                            g1nT[:, hc, :],
                            gw2[:, hc, :],
                            start=(hc == 0),
                            stop=(hc == HC - 1),
                        )
                    logits = pgs.tile([128, E], f32, tag="logits")
                    nc.vector.tensor_add(logits, ps2, gb2b)
                    # softmax over 16 experts
                    negmax = pgs.tile([128, 1], f32, tag="negmax")
                    nc.vector.tensor_reduce(
                        negmax, logits, axis=mybir.AxisListType.X, op=OP.max, negate=True
                    )
                    pexp = pgs.tile([128, E], f32, tag="pexp")
                    nc.scalar.activation(pexp, logits, AF.Exp, bias=negmax)
                    rsum = pgs.tile([128, 1], f32, tag="rsum")
                    nc.vector.reduce_sum(rsum, pexp, axis=mybir.AxisListType.X)
                    rinv = pgs.tile([128, 1], f32, tag="rinv")
                    nc.vector.reciprocal(rinv, rsum)
                    probs = pgs.tile([128, E], f32, tag="probs")
                    nc.vector.tensor_scalar_mul(probs, pexp, rinv)
                    # top-4 mask + renormalized weights
                    top8 = pgs.tile([128, 8], f32, tag="top8")
                    nc.vector.max(top8, probs)
                    mask = pgs.tile([128, E], f32, tag="mask")
                    nc.vector.tensor_scalar(
                        mask, probs, top8[:, 3:4], None, op0=OP.is_ge
                    )
                    eprob = pgs.tile([128, E], f32, tag="eprob")
                    nc.scalar.activation(eprob, probs, AF.Exp)
                    emask = pgs.tile([128, E], f32, tag="emask")
                    nc.vector.tensor_mul(emask, eprob, mask)
                    den = pgs.tile([128, 1], f32, tag="den")
                    nc.vector.reduce_sum(den, emask, axis=mybir.AxisListType.X)
                    dinv = pgs.tile([128, 1], f32, tag="dinv")
                    nc.vector.reciprocal(dinv, den)
                    w = pgs.tile([128, E], f32, tag="w")
                    nc.vector.tensor_scalar_mul(w, emask, dinv)
                    # transpose w -> wT[:, bt]
                    tpw = pgp.tile([E, 128], f32, tag="gps")
                    nc.tensor.transpose(tpw, w, ident)
                    nc.scalar.copy(wT[:, bs], tpw)

            # ================= EXPERTS (feature-major, fp32r) =================
            for e in range(E):
                h1T = ph.tile([128, HC, BS], f32r, tag="h1T")
                # layer 1: h1 = gelu(s1*(xT.T@w1) + t1), [1024 h, 512 tok]
                for q in range(4):
                    w1q = pw.tile([128, DC, 256], f32r, tag="ew1q", bufs=3)
                    nc.sync.dma_start(
                        w1q,
                        ew1_d[e, :, bass.ts(q, 256)].rearrange(
                            "(dc p) m -> p dc m", p=128
                        ),
                    )
                    for mi in range(2):
                        hidx = q * 2 + mi
                        ps = pmm.tile([128, BS], f32, tag="mm")
                        for dc in range(DC):
                            nc.tensor.matmul(
                                ps,
                                w1q[:, dc, bass.ts(mi, 128)],
                                xT[:, dc, :],
                                start=(dc == 0),
                                stop=(dc == DC - 1),
                            )
                        nc.scalar.activation(
                            h1T[:, hidx, :],
                            ps,
                            AF.Gelu,
                            bias=et1[:, e, hidx : hidx + 1],
                            scale=es1[:, e, hidx : hidx + 1],
                        )
                # routing weight broadcast: wb[p, b] = wT[e, b]
                wrow = pt.tile([1, BS], f32, tag="wrow")
                nc.sync.dma_start(wrow, wT[e : e + 1, :])
                wb = pwb.tile([128, BS], f32, tag="wb")
                nc.tensor.matmul(wb, ones_row[0:1, :], wrow[0:1, :], start=True, stop=True)
                # layer 2 + combine
                for q in range(4):
                    w2q = pw.tile([128, HC, 256], f32r, tag="ew2q", bufs=3)
                    nc.sync.dma_start(
                        w2q,
                        ew2_d[e, :, bass.ts(q, 256)].rearrange(
                            "(kc p) m -> p kc m", p=128
                        ),
                    )
                    for mi in range(2):
                        m2 = q * 2 + mi
                        ps = pmm.tile([128, BS], f32, tag="mm")
                        for kc in range(HC):
                            nc.tensor.matmul(
                                ps,
                                w2q[:, kc, bass.ts(mi, 128)],
                                h1T[:, kc, :],
                                start=(kc == 0),
                                stop=(kc == HC - 1),
                            )
                        h2 = pt.tile([128, BS], f32, tag="h2", bufs=4)
                        nc.scalar.activation(
                            h2,
                            ps,
                            AF.Gelu,
                            bias=et2[:, e, m2 : m2 + 1],
                            scale=es2[:, e, m2 : m2 + 1],
                        )
                        if e == 0:
                            nc.vector.tensor_mul(fused[:, m2, :], h2, wb)
                        else:
                            tmp = pt.tile([128, BS], f32, tag="cmb", bufs=4)
                            nc.vector.tensor_mul(tmp, h2, wb)
                            nc.vector.tensor_add(
                                fused[:, m2, :], fused[:, m2, :], tmp
                            )

            # ================= FINAL HEAD (feature-major, fp32r) =================
            fusedr = ph.tile([128, HC, BS], f32r, tag="h1T")
            nc.vector.tensor_copy(fusedr, fused)
            ffT = ph.tile([128, 4, BS], f32r, tag="h1T")
            for m in range(4):
                ps = pmm.tile([128, BS], f32, tag="mm")
                for kc in range(HC):
                    nc.tensor.matmul(
                        ps,
                        fw1[:, kc, bass.ts(m, 128)],
                        fusedr[:, kc, :],
                        start=(kc == 0),
                        stop=(kc == HC - 1),
                    )
                nc.scalar.activation(
                    ffT[:, m, :],
                    ps,
                    AF.Gelu,
                    bias=ftb[:, m : m + 1],
                    scale=fsb[:, m : m + 1],
                )
            with tc.tile_pool(name="pfp", bufs=2, space="PSUM") as pfp:
                ps = pfp.tile([C, BS], f32, tag="fo")
                for kc in range(4):
                    nc.tensor.matmul(
                        ps,
                        fw2[:, kc, :],
                        ffT[:, kc, :],
                        start=(kc == 0),
                        stop=(kc == 3),
                    )
                outT = pt.tile([C, BS], f32, tag="outT")
                nc.scalar.activation(outT, ps, AF.Identity, bias=fb2[:, 0:1])
                nc.sync.dma_start(out_d[:, :], outT)

    nc.compile()
    return nc


def prep_inputs(inputs):
    """Host-side prep: returns list of per-core input maps."""
    g = {k: np.asarray(v, dtype=np.float32) for k, v in inputs.items()}

    combined = np.concatenate([g["wifi_feat"], g["rfid_feat"]], axis=1)  # [B, D]
    xT = np.ascontiguousarray(combined.T)  # [D, B]

    def fold(b_lin, bn_g, bn_b, bn_m, bn_v):
        s = bn_g.astype(np.float64) / np.sqrt(bn_v.astype(np.float64) + EPS)
        t = (b_lin.astype(np.float64) - bn_m.astype(np.float64)) * s + bn_b.astype(
            np.float64
        )
        return s.astype(np.float32), t.astype(np.float32)

    s1, t1 = fold(g["exp_b1"], g["exp_bn1_g"], g["exp_bn1_b"], g["exp_bn1_m"], g["exp_bn1_v"])
    s2, t2 = fold(g["exp_b2"], g["exp_bn2_g"], g["exp_bn2_b"], g["exp_bn2_m"], g["exp_bn2_v"])
    fs, ft = fold(g["fin_b1"], g["fin_bn_g"], g["fin_bn_b"], g["fin_bn_m"], g["fin_bn_v"])

    def pmaj(x):  # [..., M*128] -> [..., 128, M] partition-major
        return np.ascontiguousarray(
            x.reshape(*x.shape[:-1], x.shape[-1] // 128, 128).swapaxes(-1, -2)
        )

    shared = {
        "gw1": np.ascontiguousarray(g["gate_w1"]),
        "gb1": g["gate_b1"],
        "lng": g["gate_ln_g"],
        "lnb": g["gate_ln_b"],
        "gw2": np.ascontiguousarray(g["gate_w2"]),
        "gb2": g["gate_b2"],
        "ew1": np.ascontiguousarray(g["exp_w1"]),
        "ew2": np.ascontiguousarray(g["exp_w2"]),
        "es1": pmaj(s1),
        "et1": pmaj(t1),
        "es2": pmaj(s2),
        "et2": pmaj(t2),
        "fw1": np.ascontiguousarray(g["fin_w1"]),
        "fs": pmaj(fs),
        "ft": pmaj(ft),
        "fw2": np.ascontiguousarray(g["fin_w2"]),
        "fb2": np.ascontiguousarray(g["fin_b2"].reshape(C, 1)),
    }
    per_core = []
    for c in range(NCORES):
        m = dict(shared)
        m["xT"] = np.ascontiguousarray(xT[:, c * BS : (c + 1) * BS])
        per_core.append(m)
    return per_core


_NC_CACHE = None


def kernel(**inputs) -> np.ndarray:
    global _NC_CACHE
    if _NC_CACHE is None:
        _NC_CACHE = build_nc()
    nc = _NC_CACHE
    in_maps = prep_inputs(inputs)
    res = run_bass_kernel_spmd(nc, in_maps, core_ids=list(range(NCORES)))
    out = np.concatenate(
        [np.asarray(r["outT"]).T for r in res.results], axis=0
    )  # [B, C]
    return np.ascontiguousarray(out)


# revision 9
# speedup vs baseline: 1.0933x; 1.0128x over previous
"""Trainium2 Bass kernel for nn_MoE_77644418777543.

MoE: B=4096 tokens, D=512 in, H=1024 hidden, E=16 experts (dense compute,
top-4 weighted combine), gate = Linear+LN+GELU+Linear+softmax, final head
Linear+BN+GELU+Linear.

Strategy: data-parallel over batch across 8 NeuronCores (512 tokens/core),
expert/gate/final weights replicated. No collectives. Per core:
  - gate computed token-major in fp32 (routing selection must be exact);
  - expert MLPs computed feature-major with fp32r matmuls (full PE rate,
    ~1.5e-4 precision) with BN+bias+GELU folded into ScalarE activation;
  - dense top-4 combine via per-token weight vector (zeros outside top-4)
    accumulated on VectorE;
  - final head feature-major fp32r.

Host-side prep: concat+transpose of inputs, BN folding (float64), layout
rearrangement. Output is [20, 512] per core, transposed+stacked on host.
"""

import numpy as np

import concourse.bacc as bacc
import concourse.bass as bass
import concourse.mybir as mybir
import concourse.tile as tile
from concourse.bass_utils import run_bass_kernel_spmd
from concourse.masks import make_identity

B, D, H, E, C = 4096, 512, 1024, 16, 20
NCORES = 8
BS = B // NCORES  # 512 tokens per core
EPS = 1e-5

f32 = mybir.dt.float32
f32r = mybir.dt.float32r
AF = mybir.ActivationFunctionType
OP = mybir.AluOpType

DC = D // 128  # 4 contraction tiles for D
HC = H // 128  # 8 contraction tiles for H
BT = BS // 128  # 4 token tiles per core


def build_nc():
    nc = bacc.Bacc("TRN2", target_bir_lowering=False)

    # ---- DRAM I/O ----
    xT_d = nc.dram_tensor("xT", [D, BS], f32r, kind="ExternalInput")
    gw1_d = nc.dram_tensor("gw1", [D, H], f32, kind="ExternalInput")
    gb1_d = nc.dram_tensor("gb1", [H], f32, kind="ExternalInput")
    lng_d = nc.dram_tensor("lng", [H], f32, kind="ExternalInput")
    lnb_d = nc.dram_tensor("lnb", [H], f32, kind="ExternalInput")
    gw2_d = nc.dram_tensor("gw2", [H, E], f32, kind="ExternalInput")
    gb2_d = nc.dram_tensor("gb2", [E], f32, kind="ExternalInput")
    ew1_d = nc.dram_tensor("ew1", [E, D, H], f32r, kind="ExternalInput")
    ew2_d = nc.dram_tensor("ew2", [E, H, H], f32r, kind="ExternalInput")
    es1_d = nc.dram_tensor("es1", [E, 128, HC], f32, kind="ExternalInput")
    et1_d = nc.dram_tensor("et1", [E, 128, HC], f32, kind="ExternalInput")
    es2_d = nc.dram_tensor("es2", [E, 128, HC], f32, kind="ExternalInput")
    et2_d = nc.dram_tensor("et2", [E, 128, HC], f32, kind="ExternalInput")
    fw1_d = nc.dram_tensor("fw1", [H, 512], f32r, kind="ExternalInput")
    fs_d = nc.dram_tensor("fs", [128, 4], f32, kind="ExternalInput")
    ft_d = nc.dram_tensor("ft", [128, 4], f32, kind="ExternalInput")
    fw2_d = nc.dram_tensor("fw2", [512, C], f32r, kind="ExternalInput")
    fb2_d = nc.dram_tensor("fb2", [C, 1], f32, kind="ExternalInput")
    out_d = nc.dram_tensor("outT", [C, BS], f32, kind="ExternalOutput")

    def bcast_ap(handle):
        """AP reading a 1-D dram tensor broadcast across 128 partitions."""
        ap = handle[:]
        return bass.AP(tensor=ap.tensor, offset=ap.offset, ap=[[0, 128]] + ap.ap)

    with tile.TileContext(nc) as tc:
        with (
            # big streaming pool for expert weights (created first)
            tc.tile_pool(name="pw", bufs=2) as pw,
            # persistent singles
            tc.tile_pool(name="px", bufs=1) as px,
            # expert hidden (double-buffered across experts)
            tc.tile_pool(name="ph", bufs=2) as ph,
            # gate big working tiles
            tc.tile_pool(name="pgb", bufs=1) as pgb,
            # gate small working tiles
            tc.tile_pool(name="pgs", bufs=2) as pgs,
            # small transient tiles
            tc.tile_pool(name="pt", bufs=3) as pt,
            # DRAM bounce for routing weights
            tc.tile_pool(name="pd", bufs=1, space="DRAM") as pd,
            # PSUM pools
            tc.tile_pool(name="pmm", bufs=6, space="PSUM") as pmm,
        ):
            # ---- persistent SBUF loads ----
            # split big input loads across DMA queues for a faster start
            xT = px.tile([128, DC, BS], f32r)
            xT_r = xT_d[:, :].rearrange("(dc p) b -> p dc b", p=128)
            for dc in range(DC):
                nc.sync.dma_start(xT[:, dc, :], xT_r[:, dc, :])
            xT32 = xT.bitcast(f32)

            gw1 = px.tile([128, DC, H], f32)
            gw1_r = gw1_d[:, :].rearrange("(dc p) h -> p dc h", p=128)
            for dc in range(DC):
                nc.sync.dma_start(gw1[:, dc, :], gw1_r[:, dc, :])
            gw2 = px.tile([128, HC, E], f32)
            nc.sync.dma_start(gw2, gw2_d[:, :].rearrange("(kc p) e -> p kc e", p=128))

            gb1b = px.tile([128, H], f32)
            nc.gpsimd.dma_start(gb1b, bcast_ap(gb1_d))
            lngb = px.tile([128, H], f32)
            nc.gpsimd.dma_start(lngb, bcast_ap(lng_d))
            lnbb = px.tile([128, H], f32)
            nc.gpsimd.dma_start(lnbb, bcast_ap(lnb_d))
            gb2b = px.tile([128, E], f32)
            nc.gpsimd.dma_start(gb2b, bcast_ap(gb2_d))

            es1 = px.tile([128, E, HC], f32)
            nc.sync.dma_start(es1, es1_d[:, :, :].rearrange("e p m -> p e m"))
            et1 = px.tile([128, E, HC], f32)
            nc.sync.dma_start(et1, et1_d[:, :, :].rearrange("e p m -> p e m"))
            es2 = px.tile([128, E, HC], f32)
            nc.sync.dma_start(es2, es2_d[:, :, :].rearrange("e p m -> p e m"))
            et2 = px.tile([128, E, HC], f32)
            nc.sync.dma_start(et2, et2_d[:, :, :].rearrange("e p m -> p e m"))

            fw1 = px.tile([128, HC, 512], f32r)
            nc.sync.dma_start(fw1, fw1_d[:, :].rearrange("(kc p) m -> p kc m", p=128))
            fsb = px.tile([128, 4], f32)
            nc.sync.dma_start(fsb, fs_d[:, :])
            ftb = px.tile([128, 4], f32)
            nc.sync.dma_start(ftb, ft_d[:, :])
            fw2 = px.tile([128, 4, C], f32r)
            nc.sync.dma_start(fw2, fw2_d[:, :].rearrange("(kc p) c -> p kc c", p=128))
            fb2 = px.tile([C, 1], f32)
            nc.sync.dma_start(fb2, fb2_d[:, :])

            ident = px.tile([128, 128], f32)
            make_identity(nc, ident)
            eps_t = px.tile([128, 1], f32)
            nc.vector.memset(eps_t, EPS)

            wT = px.tile([E, BS], f32)  # routing weights, expert-major
            wt_dram = pd.tile([E, BS], f32)  # DRAM copy for broadcast loads
            fused = px.tile([128, HC, BS], f32)  # combined expert output

            # ================= GATE (token-major, fp32) =================
            with tc.tile_pool(name="pgp", bufs=2, space="PSUM") as pgp:
                for bt in range(BT):
                    bs = bass.ts(bt, 128)
                    # g1 = xT.T @ gw1 + gb1  -> [128 tok, 1024 h]
                    g1 = pgb.tile([128, H], f32, tag="g1")
                    for hh in range(2):
                        ps = pmm.tile([128, 512], f32, tag="mm")
                        for dc in range(DC):
                            nc.tensor.matmul(
                                ps,
                                xT32[:, dc, bs],
                                gw1[:, dc, bass.ts(hh, 512)],
                                start=(dc == 0),
                                stop=(dc == DC - 1),
                            )
                        nc.vector.tensor_add(
                            g1[:, bass.ts(hh, 512)], ps, gb1b[:, bass.ts(hh, 512)]
                        )
                    # LayerNorm over h (free dim), in place on g1
                    stats = pgs.tile([128, 2, 6], f32, tag="stats")
                    nc.vector.bn_stats(stats[:, 0, :], g1[:, 0:512])
                    nc.vector.bn_stats(stats[:, 1, :], g1[:, 512:1024])
                    mv = pgs.tile([128, 2], f32, tag="mv")
                    nc.vector.bn_aggr(mv, stats)
                    sd = pgs.tile([128, 1], f32, tag="sd")
                    nc.scalar.activation(sd, mv[:, 1:2], AF.Sqrt, bias=eps_t)
                    rstd = pgs.tile([128, 1], f32, tag="rstd")
                    nc.vector.reciprocal(rstd, sd)
                    nc.vector.tensor_scalar(
                        g1, g1, mv[:, 0:1], rstd, op0=OP.subtract, op1=OP.mult
                    )
                    # affine + gelu
                    nc.vector.tensor_mul(g1, g1, lngb)
                    nc.vector.tensor_add(g1, g1, lnbb)
                    g1n = pgb.tile([128, H], f32, tag="g1n")
                    nc.scalar.activation(g1n, g1, AF.Gelu)
                    # transpose g1n -> feature-major g1nT (this token tile only)
                    g1nT = pgb.tile([128, HC, 128], f32, tag="g1nT")
                    for hc in range(HC):
                        tp = pgp.tile([128, 128], f32, tag="gps")
                        nc.tensor.transpose(tp, g1n[:, bass.ts(hc, 128)], ident)
                        nc.scalar.copy(g1nT[:, hc, :], tp)
                    # g2 logits = g1n @ gw2 + gb2 -> [128 tok, 16]
                    ps2 = pgp.tile([128, E], f32, tag="gps")
                    for hc in range(HC):
                        nc.tensor.matmul(
                            ps2,
                            g1nT[:, hc, :],
                            gw2[:, hc, :],
                            start=(hc == 0),
                            stop=(hc == HC - 1),
                        )
                    logits = pgs.tile([128, E], f32, tag="logits")
                    nc.vector.tensor_add(logits, ps2, gb2b)
                    # softmax over 16 experts
                    negmax = pgs.tile([128, 1], f32, tag="negmax")
                    nc.vector.tensor_reduce(
                        negmax, logits, axis=mybir.AxisListType.X, op=OP.max, negate=True
                    )
                    pexp = pgs.tile([128, E], f32, tag="pexp")
                    nc.scalar.activation(pexp, logits, AF.Exp, bias=negmax)
                    rsum = pgs.tile([128, 1], f32, tag="rsum")
                    nc.vector.reduce_sum(rsum, pexp, axis=mybir.AxisListType.X)
                    rinv = pgs.tile([128, 1], f32, tag="rinv")
                    nc.vector.reciprocal(rinv, rsum)
                    probs = pgs.tile([128, E], f32, tag="probs")
                    nc.vector.tensor_scalar_mul(probs, pexp, rinv)
                    # top-4 mask + renormalized weights
                    top8 = pgs.tile([128, 8], f32, tag="top8")
                    nc.vector.max(top8, probs)
                    mask = pgs.tile([128, E], f32, tag="mask")
                    nc.vector.tensor_scalar(
                        mask, probs, top8[:, 3:4], None, op0=OP.is_ge
                    )
                    eprob = pgs.tile([128, E], f32, tag="eprob")
                    nc.scalar.activation(eprob, probs, AF.Exp)
                    emask = pgs.tile([128, E], f32, tag="emask")
                    nc.vector.tensor_mul(emask, eprob, mask)
                    den = pgs.tile([128, 1], f32, tag="den")
                    nc.vector.reduce_sum(den, emask, axis=mybir.AxisListType.X)
                    dinv = pgs.tile([128, 1], f32, tag="dinv")
                    nc.vector.reciprocal(dinv, den)
                    w = pgs.tile([128, E], f32, tag="w")
                    nc.vector.tensor_scalar_mul(w, emask, dinv)
                    # transpose w -> wT[:, bt]
                    tpw = pgp.tile([E, 128], f32, tag="gps")
                    nc.tensor.transpose(tpw, w, ident)
                    nc.scalar.copy(wT[:, bs], tpw)

            # bounce routing weights through DRAM for partition-broadcast reads
            nc.sync.dma_start(wt_dram[:, :], wT[:, :])

            fusedr = px.tile([128, HC, BS], f32r)

            # ================= EXPERTS (feature-major, fp32r) =================
            for e in range(E):
                h1T = ph.tile([128, HC, BS], f32r, tag="h1T")
                # layer 1: h1 = gelu(s1*(xT.T@w1) + t1), [1024 h, 512 tok]
                for q in range(4):
                    w1q = pw.tile([128, DC, 256], f32r, tag="ew1q", bufs=3)
                    nc.sync.dma_start(
                        w1q,
                        ew1_d[e, :, bass.ts(q, 256)].rearrange(
                            "(dc p) m -> p dc m", p=128
                        ),
                    )
                    for mi in range(2):
                        hidx = q * 2 + mi
                        ps = pmm.tile([128, BS], f32, tag="mm")
                        for dc in range(DC):
                            nc.tensor.matmul(
                                ps,
                                w1q[:, dc, bass.ts(mi, 128)],
                                xT[:, dc, :],
                                start=(dc == 0),
                                stop=(dc == DC - 1),
                            )
                        nc.scalar.activation(
                            h1T[:, hidx, :],
                            ps,
                            AF.Gelu,
                            bias=et1[:, e, hidx : hidx + 1],
                            scale=es1[:, e, hidx : hidx + 1],
                        )
                # routing weight broadcast: wb[p, b] = wT[e, b]
                wb = pt.tile([128, BS], f32, tag="wb", bufs=2)
                wrow_ap = wt_dram[e, :]
                nc.gpsimd.dma_start(
                    wb,
                    bass.AP(
                        tensor=wrow_ap.tensor,
                        offset=wrow_ap.offset,
                        ap=[[0, 128]] + wrow_ap.ap,
                    ),
                )
                # layer 2 + combine
                for q in range(4):
                    w2q = pw.tile([128, HC, 256], f32r, tag="ew2q", bufs=3)
                    nc.sync.dma_start(
                        w2q,
                        ew2_d[e, :, bass.ts(q, 256)].rearrange(
                            "(kc p) m -> p kc m", p=128
                        ),
                    )
                    for mi in range(2):
                        m2 = q * 2 + mi
                        ps = pmm.tile([128, BS], f32, tag="mm")
                        for kc in range(HC):
                            nc.tensor.matmul(
                                ps,
                                w2q[:, kc, bass.ts(mi, 128)],
                                h1T[:, kc, :],
                                start=(kc == 0),
                                stop=(kc == HC - 1),
                            )
                        h2 = pt.tile([128, BS], f32, tag="h2", bufs=4)
                        nc.scalar.activation(
                            h2,
                            ps,
                            AF.Gelu,
                            bias=et2[:, e, m2 : m2 + 1],
                            scale=es2[:, e, m2 : m2 + 1],
                        )
                        if e == 0:
                            nc.vector.tensor_mul(fused[:, m2, :], h2, wb)
                        elif e < E - 1:
                            tmp = pt.tile([128, BS], f32, tag="cmb", bufs=4)
                            nc.vector.tensor_mul(tmp, h2, wb)
                            nc.vector.tensor_add(
                                fused[:, m2, :], fused[:, m2, :], tmp
                            )
                        else:
                            tmp = pt.tile([128, BS], f32, tag="cmb", bufs=4)
                            nc.vector.tensor_mul(tmp, h2, wb)
                            nc.vector.tensor_add(
                                fusedr[:, m2, :], fused[:, m2, :], tmp
                            )

            # ================= FINAL HEAD (feature-major, fp32r) =================
            ffT = ph.tile([128, 4, BS], f32r, tag="h1T")
            for m in range(4):
                ps = pmm.tile([128, BS], f32, tag="mm")
                for kc in range(HC):
                    nc.tensor.matmul(
                        ps,
                        fw1[:, kc, bass.ts(m, 128)],
                        fusedr[:, kc, :],
                        start=(kc == 0),
                        stop=(kc == HC - 1),
                    )
                nc.scalar.activation(
                    ffT[:, m, :],
                    ps,
                    AF.Gelu,
                    bias=ftb[:, m : m + 1],
                    scale=fsb[:, m : m + 1],
                )
            with tc.tile_pool(name="pfp", bufs=2, space="PSUM") as pfp:
                ps = pfp.tile([C, BS], f32, tag="fo")
                for kc in range(4):
                    nc.tensor.matmul(
                        ps,
                        fw2[:, kc, :],
                        ffT[:, kc, :],
                        start=(kc == 0),
                        stop=(kc == 3),
                    )
                outT = pt.tile([C, BS], f32, tag="outT")
                nc.scalar.activation(outT, ps, AF.Identity, bias=fb2[:, 0:1])
                nc.sync.dma_start(out_d[:, :], outT)

    nc.compile()
    return nc


def prep_inputs(inputs):
    """Host-side prep: returns list of per-core input maps."""
    g = {k: np.asarray(v, dtype=np.float32) for k, v in inputs.items()}

    combined = np.concatenate([g["wifi_feat"], g["rfid_feat"]], axis=1)  # [B, D]
    xT = np.ascontiguousarray(combined.T)  # [D, B]

    def fold(b_lin, bn_g, bn_b, bn_m, bn_v):
        s = bn_g.astype(np.float64) / np.sqrt(bn_v.astype(np.float64) + EPS)
        t = (b_lin.astype(np.float64) - bn_m.astype(np.float64)) * s + bn_b.astype(
            np.float64
        )
        return s.astype(np.float32), t.astype(np.float32)

    s1, t1 = fold(g["exp_b1"], g["exp_bn1_g"], g["exp_bn1_b"], g["exp_bn1_m"], g["exp_bn1_v"])
    s2, t2 = fold(g["exp_b2"], g["exp_bn2_g"], g["exp_bn2_b"], g["exp_bn2_m"], g["exp_bn2_v"])
    fs, ft = fold(g["fin_b1"], g["fin_bn_g"], g["fin_bn_b"], g["fin_bn_m"], g["fin_bn_v"])

    def pmaj(x):  # [..., M*128] -> [..., 128, M] partition-major
        return np.ascontiguousarray(
            x.reshape(*x.shape[:-1], x.shape[-1] // 128, 128).swapaxes(-1, -2)
        )

    shared = {
        "gw1": np.ascontiguousarray(g["gate_w1"]),
        "gb1": g["gate_b1"],
        "lng": g["gate_ln_g"],
        "lnb": g["gate_ln_b"],
        "gw2": np.ascontiguousarray(g["gate_w2"]),
        "gb2": g["gate_b2"],
        "ew1": np.ascontiguousarray(g["exp_w1"]),
        "ew2": np.ascontiguousarray(g["exp_w2"]),
        "es1": pmaj(s1),
        "et1": pmaj(t1),
        "es2": pmaj(s2),
        "et2": pmaj(t2),
        "fw1": np.ascontiguousarray(g["fin_w1"]),
        "fs": pmaj(fs),
        "ft": pmaj(ft),
        "fw2": np.ascontiguousarray(g["fin_w2"]),
        "fb2": np.ascontiguousarray(g["fin_b2"].reshape(C, 1)),
    }
    per_core = []
    for c in range(NCORES):
        m = dict(shared)
        m["xT"] = np.ascontiguousarray(xT[:, c * BS : (c + 1) * BS])
        per_core.append(m)
    return per_core


_NC_CACHE = None


def kernel(**inputs) -> np.ndarray:
    global _NC_CACHE
    if _NC_CACHE is None:
        _NC_CACHE = build_nc()
    nc = _NC_CACHE
    in_maps = prep_inputs(inputs)
    res = run_bass_kernel_spmd(nc, in_maps, core_ids=list(range(NCORES)))
    out = np.concatenate(
        [np.asarray(r["outT"]).T for r in res.results], axis=0
    )  # [B, C]
    return np.ascontiguousarray(out)


# revision 10
# speedup vs baseline: 1.1870x; 1.0857x over previous
"""Trainium2 Bass kernel for nn_MoE_77644418777543.

MoE: B=4096 tokens, D=512 in, H=1024 hidden, E=16 experts (dense compute,
top-4 weighted combine), gate = Linear+LN+GELU+Linear+softmax, final head
Linear+BN+GELU+Linear.

Strategy: data-parallel over batch across 8 NeuronCores (512 tokens/core),
expert/gate/final weights replicated. No collectives. Per core:
  - gate computed token-major in fp32 (routing selection must be exact);
  - expert MLPs computed feature-major with fp32r matmuls (full PE rate,
    ~1.5e-4 precision) with BN+bias+GELU folded into ScalarE activation;
  - dense top-4 combine via per-token weight vector (zeros outside top-4)
    accumulated on VectorE;
  - final head feature-major fp32r.

Host-side prep: concat+transpose of inputs, BN folding (float64), layout
rearrangement. Output is [20, 512] per core, transposed+stacked on host.
"""

import numpy as np

import concourse.bacc as bacc
import concourse.bass as bass
import concourse.mybir as mybir
import concourse.tile as tile
from concourse.bass_utils import run_bass_kernel_spmd
from concourse.masks import make_identity

B, D, H, E, C = 4096, 512, 1024, 16, 20
NCORES = 8
BS = B // NCORES  # 512 tokens per core
EPS = 1e-5

f32 = mybir.dt.float32
f32r = mybir.dt.float32r
AF = mybir.ActivationFunctionType
OP = mybir.AluOpType

DC = D // 128  # 4 contraction tiles for D
HC = H // 128  # 8 contraction tiles for H
BT = BS // 128  # 4 token tiles per core


def build_nc():
    nc = bacc.Bacc("TRN2", target_bir_lowering=False)

    # ---- DRAM I/O ----
    xT_d = nc.dram_tensor("xT", [D, BS], f32r, kind="ExternalInput")
    gw1_d = nc.dram_tensor("gw1", [D, H], f32, kind="ExternalInput")
    gb1_d = nc.dram_tensor("gb1", [H], f32, kind="ExternalInput")
    lng_d = nc.dram_tensor("lng", [H], f32, kind="ExternalInput")
    lnb_d = nc.dram_tensor("lnb", [H], f32, kind="ExternalInput")
    gw2_d = nc.dram_tensor("gw2", [H, E], f32, kind="ExternalInput")
    gb2_d = nc.dram_tensor("gb2", [E], f32, kind="ExternalInput")
    ew1_d = nc.dram_tensor("ew1", [E, D, H], f32r, kind="ExternalInput")
    ew2_d = nc.dram_tensor("ew2", [E, H, H], f32r, kind="ExternalInput")
    es1_d = nc.dram_tensor("es1", [E, 128, HC], f32, kind="ExternalInput")
    et1_d = nc.dram_tensor("et1", [E, 128, HC], f32, kind="ExternalInput")
    es2_d = nc.dram_tensor("es2", [E, 128, HC], f32, kind="ExternalInput")
    et2_d = nc.dram_tensor("et2", [E, 128, HC], f32, kind="ExternalInput")
    fw1_d = nc.dram_tensor("fw1", [H, 512], f32r, kind="ExternalInput")
    fs_d = nc.dram_tensor("fs", [128, 4], f32, kind="ExternalInput")
    ft_d = nc.dram_tensor("ft", [128, 4], f32, kind="ExternalInput")
    fw2_d = nc.dram_tensor("fw2", [512, C], f32r, kind="ExternalInput")
    fb2_d = nc.dram_tensor("fb2", [C, 1], f32, kind="ExternalInput")
    out_d = nc.dram_tensor("outT", [C, BS], f32, kind="ExternalOutput")

    def bcast_ap(handle):
        """AP reading a 1-D dram tensor broadcast across 128 partitions."""
        ap = handle[:]
        return bass.AP(tensor=ap.tensor, offset=ap.offset, ap=[[0, 128]] + ap.ap)

    with tile.TileContext(nc) as tc:
        with (
            # big streaming pool for expert weights (created first)
            tc.tile_pool(name="pw", bufs=2) as pw,
            # persistent singles
            tc.tile_pool(name="px", bufs=1) as px,
            # expert hidden (double-buffered across experts)
            tc.tile_pool(name="ph", bufs=2) as ph,
            # gate big working tiles
            tc.tile_pool(name="pgb", bufs=1) as pgb,
            # gate small working tiles
            tc.tile_pool(name="pgs", bufs=2) as pgs,
            # small transient tiles
            tc.tile_pool(name="pt", bufs=3) as pt,
            # DRAM bounce for routing weights
            tc.tile_pool(name="pd", bufs=1, space="DRAM") as pd,
            # PSUM pools
            tc.tile_pool(name="pmm", bufs=6, space="PSUM") as pmm,
        ):
            # ---- persistent SBUF loads ----
            # split big input loads across DMA queues for a faster start
            xT = px.tile([128, DC, BS], f32r)
            xT_r = xT_d[:, :].rearrange("(dc p) b -> p dc b", p=128)
            for dc in range(DC):
                for bq in range(4):
                    nc.sync.dma_start(
                        xT[:, dc, bass.ts(bq, 128)], xT_r[:, dc, bass.ts(bq, 128)]
                    )
            xT32 = xT.bitcast(f32)

            gw1 = px.tile([128, DC, H], f32)
            gw1_r = gw1_d[:, :].rearrange("(dc p) h -> p dc h", p=128)
            for dc in range(DC):
                for hq in range(4):
                    nc.sync.dma_start(
                        gw1[:, dc, bass.ts(hq, 256)], gw1_r[:, dc, bass.ts(hq, 256)]
                    )
            gw2 = px.tile([128, HC, E], f32)
            nc.sync.dma_start(gw2, gw2_d[:, :].rearrange("(kc p) e -> p kc e", p=128))

            gb1b = px.tile([128, H], f32)
            nc.gpsimd.dma_start(gb1b, bcast_ap(gb1_d))
            lngb = px.tile([128, H], f32)
            nc.gpsimd.dma_start(lngb, bcast_ap(lng_d))
            lnbb = px.tile([128, H], f32)
            nc.gpsimd.dma_start(lnbb, bcast_ap(lnb_d))
            gb2b = px.tile([128, E], f32)
            nc.gpsimd.dma_start(gb2b, bcast_ap(gb2_d))

            es1 = px.tile([128, E, HC], f32)
            nc.sync.dma_start(es1, es1_d[:, :, :].rearrange("e p m -> p e m"))
            et1 = px.tile([128, E, HC], f32)
            nc.sync.dma_start(et1, et1_d[:, :, :].rearrange("e p m -> p e m"))
            es2 = px.tile([128, E, HC], f32)
            nc.sync.dma_start(es2, es2_d[:, :, :].rearrange("e p m -> p e m"))
            et2 = px.tile([128, E, HC], f32)
            nc.sync.dma_start(et2, et2_d[:, :, :].rearrange("e p m -> p e m"))

            fw1 = px.tile([128, HC, 512], f32r)
            nc.sync.dma_start(fw1, fw1_d[:, :].rearrange("(kc p) m -> p kc m", p=128))
            fsb = px.tile([128, 4], f32)
            nc.sync.dma_start(fsb, fs_d[:, :])
            ftb = px.tile([128, 4], f32)
            nc.sync.dma_start(ftb, ft_d[:, :])
            fw2 = px.tile([128, 4, C], f32r)
            nc.sync.dma_start(fw2, fw2_d[:, :].rearrange("(kc p) c -> p kc c", p=128))
            fb2 = px.tile([C, 1], f32)
            nc.sync.dma_start(fb2, fb2_d[:, :])

            ident = px.tile([128, 128], f32)
            make_identity(nc, ident)
            eps_t = px.tile([128, 1], f32)
            nc.vector.memset(eps_t, EPS)

            wT = px.tile([E, BS], f32)  # routing weights, expert-major
            wt_dram = pd.tile([E, BS], f32)  # DRAM copy for broadcast loads
            fused = px.tile([128, HC, BS], f32)  # combined expert output

            # ================= GATE (token-major, fp32) =================
            with tc.tile_pool(name="pgp", bufs=2, space="PSUM") as pgp:
                for bt in range(BT):
                    bs = bass.ts(bt, 128)
                    # g1 = xT.T @ gw1 + gb1  -> [128 tok, 1024 h]
                    g1 = pgb.tile([128, H], f32, tag="g1")
                    for hh in range(2):
                        ps = pmm.tile([128, 512], f32, tag="mm")
                        for dc in range(DC):
                            nc.tensor.matmul(
                                ps,
                                xT32[:, dc, bs],
                                gw1[:, dc, bass.ts(hh, 512)],
                                start=(dc == 0),
                                stop=(dc == DC - 1),
                            )
                        nc.vector.tensor_add(
                            g1[:, bass.ts(hh, 512)], ps, gb1b[:, bass.ts(hh, 512)]
                        )
                    # LayerNorm over h (free dim), in place on g1
                    stats = pgs.tile([128, 2, 6], f32, tag="stats")
                    nc.vector.bn_stats(stats[:, 0, :], g1[:, 0:512])
                    nc.vector.bn_stats(stats[:, 1, :], g1[:, 512:1024])
                    mv = pgs.tile([128, 2], f32, tag="mv")
                    nc.vector.bn_aggr(mv, stats)
                    sd = pgs.tile([128, 1], f32, tag="sd")
                    nc.scalar.activation(sd, mv[:, 1:2], AF.Sqrt, bias=eps_t)
                    rstd = pgs.tile([128, 1], f32, tag="rstd")
                    nc.vector.reciprocal(rstd, sd)
                    nc.vector.tensor_scalar(
                        g1, g1, mv[:, 0:1], rstd, op0=OP.subtract, op1=OP.mult
                    )
                    # affine + gelu
                    nc.vector.tensor_mul(g1, g1, lngb)
                    nc.vector.tensor_add(g1, g1, lnbb)
                    g1n = pgb.tile([128, H], f32, tag="g1n")
                    nc.scalar.activation(g1n, g1, AF.Gelu)
                    # transpose g1n -> feature-major g1nT (this token tile only)
                    g1nT = pgb.tile([128, HC, 128], f32, tag="g1nT")
                    for hc in range(HC):
                        tp = pgp.tile([128, 128], f32, tag="gps")
                        nc.tensor.transpose(tp, g1n[:, bass.ts(hc, 128)], ident)
                        nc.scalar.copy(g1nT[:, hc, :], tp)
                    # g2 logits = g1n @ gw2 + gb2 -> [128 tok, 16]
                    ps2 = pgp.tile([128, E], f32, tag="gps")
                    for hc in range(HC):
                        nc.tensor.matmul(
                            ps2,
                            g1nT[:, hc, :],
                            gw2[:, hc, :],
                            start=(hc == 0),
                            stop=(hc == HC - 1),
                        )
                    logits = pgs.tile([128, E], f32, tag="logits")
                    nc.vector.tensor_add(logits, ps2, gb2b)
                    # softmax over 16 experts
                    negmax = pgs.tile([128, 1], f32, tag="negmax")
                    nc.vector.tensor_reduce(
                        negmax, logits, axis=mybir.AxisListType.X, op=OP.max, negate=True
                    )
                    pexp = pgs.tile([128, E], f32, tag="pexp")
                    nc.scalar.activation(pexp, logits, AF.Exp, bias=negmax)
                    rsum = pgs.tile([128, 1], f32, tag="rsum")
                    nc.vector.reduce_sum(rsum, pexp, axis=mybir.AxisListType.X)
                    rinv = pgs.tile([128, 1], f32, tag="rinv")
                    nc.vector.reciprocal(rinv, rsum)
                    probs = pgs.tile([128, E], f32, tag="probs")
                    nc.vector.tensor_scalar_mul(probs, pexp, rinv)
                    # top-4 mask + renormalized weights
                    top8 = pgs.tile([128, 8], f32, tag="top8")
                    nc.vector.max(top8, probs)
                    mask = pgs.tile([128, E], f32, tag="mask")
                    nc.vector.tensor_scalar(
                        mask, probs, top8[:, 3:4], None, op0=OP.is_ge
                    )
                    eprob = pgs.tile([128, E], f32, tag="eprob")
                    nc.scalar.activation(eprob, probs, AF.Exp)
                    emask = pgs.tile([128, E], f32, tag="emask")
                    nc.vector.tensor_mul(emask, eprob, mask)
                    den = pgs.tile([128, 1], f32, tag="den")
                    nc.vector.reduce_sum(den, emask, axis=mybir.AxisListType.X)
                    dinv = pgs.tile([128, 1], f32, tag="dinv")
                    nc.vector.reciprocal(dinv, den)
                    w = pgs.tile([128, E], f32, tag="w")
                    nc.vector.tensor_scalar_mul(w, emask, dinv)
                    # transpose w -> wT[:, bt]
                    tpw = pgp.tile([E, 128], f32, tag="gps")
                    nc.tensor.transpose(tpw, w, ident)
                    nc.scalar.copy(wT[:, bs], tpw)

            # bounce routing weights through DRAM for partition-broadcast reads
            nc.sync.dma_start(wt_dram[:, :], wT[:, :])

            fusedr = px.tile([128, HC, BS], f32r)

            # ================= EXPERTS (feature-major, fp32r) =================
            for e in range(E):
                h1T = ph.tile([128, HC, BS], f32r, tag="h1T")
                # layer 1: h1 = gelu(s1*(xT.T@w1) + t1), [1024 h, 512 tok]
                for q in range(4):
                    w1q = pw.tile([128, DC, 256], f32r, tag="ew1q", bufs=3)
                    nc.sync.dma_start(
                        w1q,
                        ew1_d[e, :, bass.ts(q, 256)].rearrange(
                            "(dc p) m -> p dc m", p=128
                        ),
                    )
                    for mi in range(2):
                        hidx = q * 2 + mi
                        ps = pmm.tile([128, BS], f32, tag="mm")
                        for dc in range(DC):
                            nc.tensor.matmul(
                                ps,
                                w1q[:, dc, bass.ts(mi, 128)],
                                xT[:, dc, :],
                                start=(dc == 0),
                                stop=(dc == DC - 1),
                            )
                        nc.scalar.activation(
                            h1T[:, hidx, :],
                            ps,
                            AF.Gelu,
                            bias=et1[:, e, hidx : hidx + 1],
                            scale=es1[:, e, hidx : hidx + 1],
                        )
                # routing weight broadcast: wb[p, b] = wT[e, b]
                wb = pt.tile([128, BS], f32, tag="wb", bufs=4)
                wrow_ap = wt_dram[e, :]
                nc.sync.dma_start(
                    wb,
                    bass.AP(
                        tensor=wrow_ap.tensor,
                        offset=wrow_ap.offset,
                        ap=[[0, 128]] + wrow_ap.ap,
                    ),
                )
                # layer 2 + combine
                for q in range(4):
                    w2q = pw.tile([128, HC, 256], f32r, tag="ew2q", bufs=3)
                    nc.sync.dma_start(
                        w2q,
                        ew2_d[e, :, bass.ts(q, 256)].rearrange(
                            "(kc p) m -> p kc m", p=128
                        ),
                    )
                    for mi in range(2):
                        m2 = q * 2 + mi
                        ps = pmm.tile([128, BS], f32, tag="mm")
                        for kc in range(HC):
                            nc.tensor.matmul(
                                ps,
                                w2q[:, kc, bass.ts(mi, 128)],
                                h1T[:, kc, :],
                                start=(kc == 0),
                                stop=(kc == HC - 1),
                            )
                        h2 = pt.tile([128, BS], f32, tag="h2", bufs=4)
                        nc.scalar.activation(
                            h2,
                            ps,
                            AF.Gelu,
                            bias=et2[:, e, m2 : m2 + 1],
                            scale=es2[:, e, m2 : m2 + 1],
                        )
                        if e == 0:
                            nc.vector.tensor_mul(fused[:, m2, :], h2, wb)
                        elif e < E - 1:
                            tmp = pt.tile([128, BS], f32, tag="cmb", bufs=4)
                            nc.vector.tensor_mul(tmp, h2, wb)
                            nc.vector.tensor_add(
                                fused[:, m2, :], fused[:, m2, :], tmp
                            )
                        else:
                            tmp = pt.tile([128, BS], f32, tag="cmb", bufs=4)
                            nc.vector.tensor_mul(tmp, h2, wb)
                            nc.vector.tensor_add(
                                fusedr[:, m2, :], fused[:, m2, :], tmp
                            )

            # ================= FINAL HEAD (feature-major, fp32r) =================
            ffT = ph.tile([128, 4, BS], f32r, tag="h1T")
            for m in range(4):
                ps = pmm.tile([128, BS], f32, tag="mm")
                for kc in range(HC):
                    nc.tensor.matmul(
                        ps,
                        fw1[:, kc, bass.ts(m, 128)],
                        fusedr[:, kc, :],
                        start=(kc == 0),
                        stop=(kc == HC - 1),
                    )
                nc.scalar.activation(
                    ffT[:, m, :],
                    ps,
                    AF.Gelu,
                    bias=ftb[:, m : m + 1],
                    scale=fsb[:, m : m + 1],
                )
            with tc.tile_pool(name="pfp", bufs=2, space="PSUM") as pfp:
                ps = pfp.tile([C, BS], f32, tag="fo")
                for kc in range(4):
                    nc.tensor.matmul(
                        ps,
                        fw2[:, kc, :],
                        ffT[:, kc, :],
                        start=(kc == 0),
                        stop=(kc == 3),
                    )
                outT = pt.tile([C, BS], f32, tag="outT")
                nc.scalar.activation(outT, ps, AF.Identity, bias=fb2[:, 0:1])
                nc.sync.dma_start(out_d[:, :], outT)

    nc.compile()
    return nc


def prep_inputs(inputs):
    """Host-side prep: returns list of per-core input maps."""
    g = {k: np.asarray(v, dtype=np.float32) for k, v in inputs.items()}

    combined = np.concatenate([g["wifi_feat"], g["rfid_feat"]], axis=1)  # [B, D]
    xT = np.ascontiguousarray(combined.T)  # [D, B]

    def fold(b_lin, bn_g, bn_b, bn_m, bn_v):
        s = bn_g.astype(np.float64) / np.sqrt(bn_v.astype(np.float64) + EPS)
        t = (b_lin.astype(np.float64) - bn_m.astype(np.float64)) * s + bn_b.astype(
            np.float64
        )
        return s.astype(np.float32), t.astype(np.float32)

    s1, t1 = fold(g["exp_b1"], g["exp_bn1_g"], g["exp_bn1_b"], g["exp_bn1_m"], g["exp_bn1_v"])
    s2, t2 = fold(g["exp_b2"], g["exp_bn2_g"], g["exp_bn2_b"], g["exp_bn2_m"], g["exp_bn2_v"])
    fs, ft = fold(g["fin_b1"], g["fin_bn_g"], g["fin_bn_b"], g["fin_bn_m"], g["fin_bn_v"])

    def pmaj(x):  # [..., M*128] -> [..., 128, M] partition-major
        return np.ascontiguousarray(
            x.reshape(*x.shape[:-1], x.shape[-1] // 128, 128).swapaxes(-1, -2)
        )

    shared = {
        "gw1": np.ascontiguousarray(g["gate_w1"]),
        "gb1": g["gate_b1"],
        "lng": g["gate_ln_g"],
        "lnb": g["gate_ln_b"],
        "gw2": np.ascontiguousarray(g["gate_w2"]),
        "gb2": g["gate_b2"],
        "ew1": np.ascontiguousarray(g["exp_w1"]),
        "ew2": np.ascontiguousarray(g["exp_w2"]),
        "es1": pmaj(s1),
        "et1": pmaj(t1),
        "es2": pmaj(s2),
        "et2": pmaj(t2),
        "fw1": np.ascontiguousarray(g["fin_w1"]),
        "fs": pmaj(fs),
        "ft": pmaj(ft),
        "fw2": np.ascontiguousarray(g["fin_w2"]),
        "fb2": np.ascontiguousarray(g["fin_b2"].reshape(C, 1)),
    }
    per_core = []
    for c in range(NCORES):
        m = dict(shared)
        m["xT"] = np.ascontiguousarray(xT[:, c * BS : (c + 1) * BS])
        per_core.append(m)
    return per_core


_NC_CACHE = None


def kernel(**inputs) -> np.ndarray:
    global _NC_CACHE
    if _NC_CACHE is None:
        _NC_CACHE = build_nc()
    nc = _NC_CACHE
    in_maps = prep_inputs(inputs)
    res = run_bass_kernel_spmd(nc, in_maps, core_ids=list(range(NCORES)))
    out = np.concatenate(
        [np.asarray(r["outT"]).T for r in res.results], axis=0
    )  # [B, C]
    return np.ascontiguousarray(out)


# revision 11
# speedup vs baseline: 1.2151x; 1.0237x over previous
"""Trainium2 Bass kernel for nn_MoE_77644418777543.

MoE: B=4096 tokens, D=512 in, H=1024 hidden, E=16 experts (dense compute,
top-4 weighted combine), gate = Linear+LN+GELU+Linear+softmax, final head
Linear+BN+GELU+Linear.

Strategy: data-parallel over batch across 8 NeuronCores (512 tokens/core),
expert/gate/final weights replicated. No collectives. Per core:
  - gate computed token-major in fp32 (routing selection must be exact);
  - expert MLPs computed feature-major with fp32r matmuls (full PE rate,
    ~1.5e-4 precision) with BN+bias+GELU folded into ScalarE activation;
  - dense top-4 combine via per-token weight vector (zeros outside top-4)
    accumulated on VectorE;
  - final head feature-major fp32r.

Host-side prep: concat+transpose of inputs, BN folding (float64), layout
rearrangement. Output is [20, 512] per core, transposed+stacked on host.
"""

import numpy as np

import concourse.bacc as bacc
import concourse.bass as bass
import concourse.mybir as mybir
import concourse.tile as tile
from concourse.bass_utils import run_bass_kernel_spmd
from concourse.masks import make_identity

B, D, H, E, C = 4096, 512, 1024, 16, 20
NCORES = 8
BS = B // NCORES  # 512 tokens per core
EPS = 1e-5

f32 = mybir.dt.float32
f32r = mybir.dt.float32r
AF = mybir.ActivationFunctionType
OP = mybir.AluOpType

DC = D // 128  # 4 contraction tiles for D
HC = H // 128  # 8 contraction tiles for H
BT = BS // 128  # 4 token tiles per core


def build_nc():
    nc = bacc.Bacc("TRN2", target_bir_lowering=False)

    # ---- DRAM I/O ----
    xT_d = nc.dram_tensor("xT", [D, BS], f32r, kind="ExternalInput")
    gw1_d = nc.dram_tensor("gw1", [D, H], f32, kind="ExternalInput")
    gb1_d = nc.dram_tensor("gb1", [H], f32, kind="ExternalInput")
    lng_d = nc.dram_tensor("lng", [H], f32, kind="ExternalInput")
    lnb_d = nc.dram_tensor("lnb", [H], f32, kind="ExternalInput")
    gw2_d = nc.dram_tensor("gw2", [H, E], f32, kind="ExternalInput")
    gb2_d = nc.dram_tensor("gb2", [E], f32, kind="ExternalInput")
    ew1_d = nc.dram_tensor("ew1", [E, D, H], f32r, kind="ExternalInput")
    ew2_d = nc.dram_tensor("ew2", [E, H, H], f32r, kind="ExternalInput")
    es1_d = nc.dram_tensor("es1", [E, 128, HC], f32, kind="ExternalInput")
    et1_d = nc.dram_tensor("et1", [E, 128, HC], f32, kind="ExternalInput")
    es2_d = nc.dram_tensor("es2", [E, 128, HC], f32, kind="ExternalInput")
    et2_d = nc.dram_tensor("et2", [E, 128, HC], f32, kind="ExternalInput")
    fw1_d = nc.dram_tensor("fw1", [H, 512], f32r, kind="ExternalInput")
    fs_d = nc.dram_tensor("fs", [128, 4], f32, kind="ExternalInput")
    ft_d = nc.dram_tensor("ft", [128, 4], f32, kind="ExternalInput")
    fw2_d = nc.dram_tensor("fw2", [512, C], f32r, kind="ExternalInput")
    fb2_d = nc.dram_tensor("fb2", [C, 1], f32, kind="ExternalInput")
    out_d = nc.dram_tensor("outT", [C, BS], f32, kind="ExternalOutput")

    def bcast_ap(handle):
        """AP reading a 1-D dram tensor broadcast across 128 partitions."""
        ap = handle[:]
        return bass.AP(tensor=ap.tensor, offset=ap.offset, ap=[[0, 128]] + ap.ap)

    with tile.TileContext(nc) as tc:
        with (
            # big streaming pool for expert weights (created first)
            tc.tile_pool(name="pw", bufs=2) as pw,
            # persistent singles
            tc.tile_pool(name="px", bufs=1) as px,
            # expert hidden (double-buffered across experts)
            tc.tile_pool(name="ph", bufs=2) as ph,
            # gate big working tiles
            tc.tile_pool(name="pgb", bufs=1) as pgb,
            # gate small working tiles
            tc.tile_pool(name="pgs", bufs=2) as pgs,
            # small transient tiles
            tc.tile_pool(name="pt", bufs=3) as pt,
            # DRAM bounce for routing weights
            tc.tile_pool(name="pd", bufs=1, space="DRAM") as pd,
            # PSUM pools
            tc.tile_pool(name="pmm", bufs=6, space="PSUM") as pmm,
        ):
            # ---- persistent SBUF loads ----
            # split big input loads across DMA queues for a faster start
            xT = px.tile([128, DC, BS], f32r)
            xT_r = xT_d[:, :].rearrange("(dc p) b -> p dc b", p=128)
            for dc in range(DC):
                for bq in range(4):
                    nc.sync.dma_start(
                        xT[:, dc, bass.ts(bq, 128)], xT_r[:, dc, bass.ts(bq, 128)]
                    )
            xT32 = xT.bitcast(f32)

            gw1 = px.tile([128, DC, H], f32)
            gw1_r = gw1_d[:, :].rearrange("(dc p) h -> p dc h", p=128)
            for dc in range(DC):
                for hq in range(4):
                    nc.sync.dma_start(
                        gw1[:, dc, bass.ts(hq, 256)], gw1_r[:, dc, bass.ts(hq, 256)]
                    )
            gw2 = px.tile([128, HC, E], f32)
            nc.sync.dma_start(gw2, gw2_d[:, :].rearrange("(kc p) e -> p kc e", p=128))

            gb1b = px.tile([128, H], f32)
            nc.gpsimd.dma_start(gb1b, bcast_ap(gb1_d))
            lngb = px.tile([128, H], f32)
            nc.gpsimd.dma_start(lngb, bcast_ap(lng_d))
            lnbb = px.tile([128, H], f32)
            nc.gpsimd.dma_start(lnbb, bcast_ap(lnb_d))
            gb2b = px.tile([128, E], f32)
            nc.gpsimd.dma_start(gb2b, bcast_ap(gb2_d))

            es1 = px.tile([128, E, HC], f32)
            nc.sync.dma_start(es1, es1_d[:, :, :].rearrange("e p m -> p e m"))
            et1 = px.tile([128, E, HC], f32)
            nc.sync.dma_start(et1, et1_d[:, :, :].rearrange("e p m -> p e m"))
            es2 = px.tile([128, E, HC], f32)
            nc.sync.dma_start(es2, es2_d[:, :, :].rearrange("e p m -> p e m"))
            et2 = px.tile([128, E, HC], f32)
            nc.sync.dma_start(et2, et2_d[:, :, :].rearrange("e p m -> p e m"))

            fw1 = px.tile([128, HC, 512], f32r)
            nc.sync.dma_start(fw1, fw1_d[:, :].rearrange("(kc p) m -> p kc m", p=128))
            fsb = px.tile([128, 4], f32)
            nc.sync.dma_start(fsb, fs_d[:, :])
            ftb = px.tile([128, 4], f32)
            nc.sync.dma_start(ftb, ft_d[:, :])
            fw2 = px.tile([128, 4, C], f32r)
            nc.sync.dma_start(fw2, fw2_d[:, :].rearrange("(kc p) c -> p kc c", p=128))
            fb2 = px.tile([C, 1], f32)
            nc.sync.dma_start(fb2, fb2_d[:, :])

            ident = px.tile([128, 128], f32)
            make_identity(nc, ident)
            eps_t = px.tile([128, 1], f32)
            nc.vector.memset(eps_t, EPS)

            wT = px.tile([E, BS], f32)  # routing weights, expert-major
            wt_dram = pd.tile([E, BS], f32)  # DRAM copy for broadcast loads
            fused = px.tile([128, HC, BS], f32)  # combined expert output

            # ================= GATE (token-major, fp32) =================
            with tc.tile_pool(name="pgp", bufs=2, space="PSUM") as pgp:
                for bt in range(BT):
                    bs = bass.ts(bt, 128)
                    # g1 = xT.T @ gw1 + gb1  -> [128 tok, 1024 h]
                    g1 = pgb.tile([128, H], f32, tag="g1")
                    for hh in range(2):
                        ps = pmm.tile([128, 512], f32, tag="mm")
                        for dc in range(DC):
                            nc.tensor.matmul(
                                ps,
                                xT32[:, dc, bs],
                                gw1[:, dc, bass.ts(hh, 512)],
                                start=(dc == 0),
                                stop=(dc == DC - 1),
                            )
                        nc.vector.tensor_add(
                            g1[:, bass.ts(hh, 512)], ps, gb1b[:, bass.ts(hh, 512)]
                        )
                    # LayerNorm over h (free dim), in place on g1
                    stats = pgs.tile([128, 2, 6], f32, tag="stats")
                    nc.vector.bn_stats(stats[:, 0, :], g1[:, 0:512])
                    nc.vector.bn_stats(stats[:, 1, :], g1[:, 512:1024])
                    mv = pgs.tile([128, 2], f32, tag="mv")
                    nc.vector.bn_aggr(mv, stats)
                    sd = pgs.tile([128, 1], f32, tag="sd")
                    nc.scalar.activation(sd, mv[:, 1:2], AF.Sqrt, bias=eps_t)
                    rstd = pgs.tile([128, 1], f32, tag="rstd")
                    nc.vector.reciprocal(rstd, sd)
                    nc.vector.tensor_scalar(
                        g1, g1, mv[:, 0:1], rstd, op0=OP.subtract, op1=OP.mult
                    )
                    # affine + gelu
                    nc.vector.tensor_mul(g1, g1, lngb)
                    nc.vector.tensor_add(g1, g1, lnbb)
                    g1n = pgb.tile([128, H], f32, tag="g1n")
                    nc.scalar.activation(g1n, g1, AF.Gelu)
                    # transpose g1n -> feature-major g1nT (this token tile only)
                    g1nT = pgb.tile([128, HC, 128], f32, tag="g1nT")
                    for hc in range(HC):
                        tp = pgp.tile([128, 128], f32, tag="gps")
                        nc.tensor.transpose(tp, g1n[:, bass.ts(hc, 128)], ident)
                        nc.scalar.copy(g1nT[:, hc, :], tp)
                    # g2 logits = g1n @ gw2 + gb2 -> [128 tok, 16]
                    ps2 = pgp.tile([128, E], f32, tag="gps")
                    for hc in range(HC):
                        nc.tensor.matmul(
                            ps2,
                            g1nT[:, hc, :],
                            gw2[:, hc, :],
                            start=(hc == 0),
                            stop=(hc == HC - 1),
                        )
                    logits = pgs.tile([128, E], f32, tag="logits")
                    nc.vector.tensor_add(logits, ps2, gb2b)
                    # softmax over 16 experts
                    negmax = pgs.tile([128, 1], f32, tag="negmax")
                    nc.vector.tensor_reduce(
                        negmax, logits, axis=mybir.AxisListType.X, op=OP.max, negate=True
                    )
                    pexp = pgs.tile([128, E], f32, tag="pexp")
                    nc.scalar.activation(pexp, logits, AF.Exp, bias=negmax)
                    rsum = pgs.tile([128, 1], f32, tag="rsum")
                    nc.vector.reduce_sum(rsum, pexp, axis=mybir.AxisListType.X)
                    rinv = pgs.tile([128, 1], f32, tag="rinv")
                    nc.vector.reciprocal(rinv, rsum)
                    probs = pgs.tile([128, E], f32, tag="probs")
                    nc.vector.tensor_scalar_mul(probs, pexp, rinv)
                    # top-4 mask + renormalized weights
                    top8 = pgs.tile([128, 8], f32, tag="top8")
                    nc.vector.max(top8, probs)
                    mask = pgs.tile([128, E], f32, tag="mask")
                    nc.vector.tensor_scalar(
                        mask, probs, top8[:, 3:4], None, op0=OP.is_ge
                    )
                    eprob = pgs.tile([128, E], f32, tag="eprob")
                    nc.scalar.activation(eprob, probs, AF.Exp)
                    emask = pgs.tile([128, E], f32, tag="emask")
                    nc.vector.tensor_mul(emask, eprob, mask)
                    den = pgs.tile([128, 1], f32, tag="den")
                    nc.vector.reduce_sum(den, emask, axis=mybir.AxisListType.X)
                    dinv = pgs.tile([128, 1], f32, tag="dinv")
                    nc.vector.reciprocal(dinv, den)
                    w = pgs.tile([128, E], f32, tag="w")
                    nc.vector.tensor_scalar_mul(w, emask, dinv)
                    # transpose w -> wT[:, bt]
                    tpw = pgp.tile([E, 128], f32, tag="gps")
                    nc.tensor.transpose(tpw, w, ident)
                    nc.scalar.copy(wT[:, bs], tpw)

            # bounce routing weights through DRAM for partition-broadcast reads
            nc.sync.dma_start(wt_dram[:, :], wT[:, :])

            fusedr = px.tile([128, HC, BS], f32r)

            # ================= EXPERTS (feature-major, fp32r) =================
            for e in range(E):
                h1T = ph.tile([128, HC, BS], f32r, tag="h1T")
                # layer 1: h1 = gelu(s1*(xT.T@w1) + t1), [1024 h, 512 tok]
                for q in range(4):
                    w1q = pw.tile([128, DC, 256], f32r, tag="ew1q", bufs=3)
                    w1q_r = ew1_d[e, :, bass.ts(q, 256)].rearrange(
                        "(dc p) m -> p dc m", p=128
                    )
                    for sub in range(2):
                        nc.sync.dma_start(
                            w1q[:, bass.ts(sub, 2), :], w1q_r[:, bass.ts(sub, 2), :]
                        )
                    for mi in range(2):
                        hidx = q * 2 + mi
                        ps = pmm.tile([128, BS], f32, tag="mm")
                        for dc in range(DC):
                            nc.tensor.matmul(
                                ps,
                                w1q[:, dc, bass.ts(mi, 128)],
                                xT[:, dc, :],
                                start=(dc == 0),
                                stop=(dc == DC - 1),
                            )
                        nc.scalar.activation(
                            h1T[:, hidx, :],
                            ps,
                            AF.Gelu,
                            bias=et1[:, e, hidx : hidx + 1],
                            scale=es1[:, e, hidx : hidx + 1],
                        )
                # routing weight broadcast: wb[p, b] = wT[e, b]
                wb = pt.tile([128, BS], f32, tag="wb", bufs=4)
                wrow_ap = wt_dram[e, :]
                nc.sync.dma_start(
                    wb,
                    bass.AP(
                        tensor=wrow_ap.tensor,
                        offset=wrow_ap.offset,
                        ap=[[0, 128]] + wrow_ap.ap,
                    ),
                )
                # layer 2 + combine
                for q in range(4):
                    w2q = pw.tile([128, HC, 256], f32r, tag="ew2q", bufs=3)
                    w2q_r = ew2_d[e, :, bass.ts(q, 256)].rearrange(
                        "(kc p) m -> p kc m", p=128
                    )
                    for sub in range(4):
                        nc.sync.dma_start(
                            w2q[:, bass.ts(sub, 2), :], w2q_r[:, bass.ts(sub, 2), :]
                        )
                    for mi in range(2):
                        m2 = q * 2 + mi
                        ps = pmm.tile([128, BS], f32, tag="mm")
                        for kc in range(HC):
                            nc.tensor.matmul(
                                ps,
                                w2q[:, kc, bass.ts(mi, 128)],
                                h1T[:, kc, :],
                                start=(kc == 0),
                                stop=(kc == HC - 1),
                            )
                        h2 = pt.tile([128, BS], f32, tag="h2", bufs=4)
                        nc.scalar.activation(
                            h2,
                            ps,
                            AF.Gelu,
                            bias=et2[:, e, m2 : m2 + 1],
                            scale=es2[:, e, m2 : m2 + 1],
                        )
                        if e == 0:
                            nc.vector.tensor_mul(fused[:, m2, :], h2, wb)
                        elif e < E - 1:
                            tmp = pt.tile([128, BS], f32, tag="cmb", bufs=4)
                            nc.vector.tensor_mul(tmp, h2, wb)
                            nc.vector.tensor_add(
                                fused[:, m2, :], fused[:, m2, :], tmp
                            )
                        else:
                            tmp = pt.tile([128, BS], f32, tag="cmb", bufs=4)
                            nc.vector.tensor_mul(tmp, h2, wb)
                            nc.vector.tensor_add(
                                fusedr[:, m2, :], fused[:, m2, :], tmp
                            )

            # ================= FINAL HEAD (feature-major, fp32r) =================
            ffT = ph.tile([128, 4, BS], f32r, tag="h1T")
            for m in range(4):
                ps = pmm.tile([128, BS], f32, tag="mm")
                for kc in range(HC):
                    nc.tensor.matmul(
                        ps,
                        fw1[:, kc, bass.ts(m, 128)],
                        fusedr[:, kc, :],
                        start=(kc == 0),
                        stop=(kc == HC - 1),
                    )
                nc.scalar.activation(
                    ffT[:, m, :],
                    ps,
                    AF.Gelu,
                    bias=ftb[:, m : m + 1],
                    scale=fsb[:, m : m + 1],
                )
            with tc.tile_pool(name="pfp", bufs=2, space="PSUM") as pfp:
                ps = pfp.tile([C, BS], f32, tag="fo")
                for kc in range(4):
                    nc.tensor.matmul(
                        ps,
                        fw2[:, kc, :],
                        ffT[:, kc, :],
                        start=(kc == 0),
                        stop=(kc == 3),
                    )
                outT = pt.tile([C, BS], f32, tag="outT")
                nc.scalar.activation(outT, ps, AF.Identity, bias=fb2[:, 0:1])
                nc.sync.dma_start(out_d[:, :], outT)

    nc.compile()
    return nc


def prep_inputs(inputs):
    """Host-side prep: returns list of per-core input maps."""
    g = {k: np.asarray(v, dtype=np.float32) for k, v in inputs.items()}

    combined = np.concatenate([g["wifi_feat"], g["rfid_feat"]], axis=1)  # [B, D]
    xT = np.ascontiguousarray(combined.T)  # [D, B]

    def fold(b_lin, bn_g, bn_b, bn_m, bn_v):
        s = bn_g.astype(np.float64) / np.sqrt(bn_v.astype(np.float64) + EPS)
        t = (b_lin.astype(np.float64) - bn_m.astype(np.float64)) * s + bn_b.astype(
            np.float64
        )
        return s.astype(np.float32), t.astype(np.float32)

    s1, t1 = fold(g["exp_b1"], g["exp_bn1_g"], g["exp_bn1_b"], g["exp_bn1_m"], g["exp_bn1_v"])
    s2, t2 = fold(g["exp_b2"], g["exp_bn2_g"], g["exp_bn2_b"], g["exp_bn2_m"], g["exp_bn2_v"])
    fs, ft = fold(g["fin_b1"], g["fin_bn_g"], g["fin_bn_b"], g["fin_bn_m"], g["fin_bn_v"])

    def pmaj(x):  # [..., M*128] -> [..., 128, M] partition-major
        return np.ascontiguousarray(
            x.reshape(*x.shape[:-1], x.shape[-1] // 128, 128).swapaxes(-1, -2)
        )

    shared = {
        "gw1": np.ascontiguousarray(g["gate_w1"]),
        "gb1": g["gate_b1"],
        "lng": g["gate_ln_g"],
        "lnb": g["gate_ln_b"],
        "gw2": np.ascontiguousarray(g["gate_w2"]),
        "gb2": g["gate_b2"],
        "ew1": np.ascontiguousarray(g["exp_w1"]),
        "ew2": np.ascontiguousarray(g["exp_w2"]),
        "es1": pmaj(s1),
        "et1": pmaj(t1),
        "es2": pmaj(s2),
        "et2": pmaj(t2),
        "fw1": np.ascontiguousarray(g["fin_w1"]),
        "fs": pmaj(fs),
        "ft": pmaj(ft),
        "fw2": np.ascontiguousarray(g["fin_w2"]),
        "fb2": np.ascontiguousarray(g["fin_b2"].reshape(C, 1)),
    }
    per_core = []
    for c in range(NCORES):
        m = dict(shared)
        m["xT"] = np.ascontiguousarray(xT[:, c * BS : (c + 1) * BS])
        per_core.append(m)
    return per_core


_NC_CACHE = None


def kernel(**inputs) -> np.ndarray:
    global _NC_CACHE
    if _NC_CACHE is None:
        _NC_CACHE = build_nc()
    nc = _NC_CACHE
    in_maps = prep_inputs(inputs)
    res = run_bass_kernel_spmd(nc, in_maps, core_ids=list(range(NCORES)))
    out = np.concatenate(
        [np.asarray(r["outT"]).T for r in res.results], axis=0
    )  # [B, C]
    return np.ascontiguousarray(out)
